# revision 24
# baseline (speedup 1.0000x reference)
import sys
sys.path.insert(0, '/opt/trn_rl_repo')
import numpy as np
import ml_dtypes
import concourse.bacc as bacc
import concourse.mybir as mybir
import concourse.tile as tile
from concourse.bass_utils import run_bass_kernel_spmd

F32 = mybir.dt.float32
BF16 = mybir.dt.bfloat16
ALU = mybir.AluOpType
ACTF = mybir.ActivationFunctionType

B, T, H, O = 16, 2048, 512, 512
NB = 2            # batch rows per core
NCORES = 8
NMT = T // 512    # 512-token tiles per row
SCH = 1024        # scan chunk length (= half of T)
LN_EPS = 1e-6

_CACHE = {}

# cst column layout: 3 consts x 4 blocks
C_BRG, C_BIG, C_CRCI = range(3)


def _build():
    nc = bacc.Bacc(None, target_bir_lowering=False)
    xin = nc.declare_dram_parameter("x_t", [NB, H, T], BF16, False)
    Brg = nc.declare_dram_parameter("Brg", [H, H], BF16, False)
    Big = nc.declare_dram_parameter("Big", [H, H], BF16, False)
    Crt = nc.declare_dram_parameter("Crt", [H, H], BF16, False)
    Cin = nc.declare_dram_parameter("Cin", [H, H], BF16, False)
    W12 = nc.declare_dram_parameter("W12", [H, H], BF16, False)
    TABS = nc.declare_dram_parameter("tabs", [8 * 128, T], BF16, False)
    RHO = nc.declare_dram_parameter("rho", [4 * 128, SCH], F32, False)
    CST = nc.declare_dram_parameter("cst", [128, 4 * 3], F32, False)
    out = nc.declare_dram_parameter("out_t", [NB, T, O], F32, True)

    with tile.TileContext(nc) as tc:
        with tc.tile_pool(name="wpool", bufs=1) as wp, \
             tc.tile_pool(name="upool", bufs=1) as up, \
             tc.tile_pool(name="tmpp", bufs=1) as tp, \
             tc.tile_pool(name="xp", bufs=5) as xp, \
             tc.tile_pool(name="yp", bufs=8) as yp, \
             tc.tile_pool(name="y2p", bufs=4) as y2p, \
             tc.tile_pool(name="stp", bufs=1) as stp, \
             tc.tile_pool(name="a1p", bufs=2) as a1p, \
             tc.tile_pool(name="ofp", bufs=3) as ofp, \
             tc.tile_pool(name="ps_mm1", bufs=2, space="PSUM") as ps1, \
             tc.tile_pool(name="ps_y", bufs=2, space="PSUM") as psy, \
             tc.tile_pool(name="ps_st", bufs=1, space="PSUM") as pst, \
             tc.tile_pool(name="ps_p4", bufs=2, space="PSUM") as ps4:

            # ---- early weights (mm1 path) ----
            brg_t = wp.tile([128, 4 * 512], BF16, tag="brg")
            big_t = wp.tile([128, 4 * 512], BF16, tag="big")
            cst_t = wp.tile([128, 4 * 3], F32, tag="cst")
            for (dst, src) in ((brg_t, Brg), (big_t, Big)):
                nc.sync.dma_start(
                    out=dst[:].rearrange("p (k n) -> p k n", k=4),
                    in_=src[:].rearrange("(k p) n -> p k n", p=128))
            nc.sync.dma_start(out=cst_t[:], in_=CST[:])

            cr_t = wp.tile([128, 4 * 512], BF16, tag="cr")
            ci_t = wp.tile([128, 4 * 512], BF16, tag="ci")
            w12_t = wp.tile([128, 4 * 512], BF16, tag="w12")
            tab_t = wp.tile([128, 8 * T], BF16, tag="tabs")
            rho_t = wp.tile([128, 4 * SCH], F32, tag="rho")
            ones_t = wp.tile([128, 128], BF16, tag="ones")
            ones32 = wp.tile([1, 1], F32, tag="ones32")
            eps_t = wp.tile([128, 1], F32, tag="eps")
            nc.vector.memset(ones_t[:], 1.0)
            nc.vector.memset(ones32[:], 1.0)
            nc.vector.memset(eps_t[:], LN_EPS)

            def col(c, blk):
                return cst_t[:, c * 4 + blk:c * 4 + blk + 1]

            def ctab(bk):
                return tab_t[:, (2 * bk) * T:(2 * bk + 1) * T]

            def stab(bk):
                return tab_t[:, (2 * bk + 1) * T:(2 * bk + 2) * T]

            # u/h storage: per (b, Bk): R and I planes, token-contiguous
            U = up.tile([128, NB * 4 * 2 * T], BF16, tag="u")
            uv = U[:].rearrange("p (b k c t) -> p b k c t", b=NB, k=4, c=2)

            def uplane(b, bk, c):
                return uv[:, b:b + 1, bk:bk + 1, c:c + 1, :].squeeze()

            tmps = [[tp.tile([128, T], BF16, tag=f"tmp{j}_{g}", name=f"tmp{j}_{g}")
                     for j in range(3)] for g in range(2)]
            carry = tp.tile([128, 8], BF16, tag="carry")

            def mm1(b):
                xts = []
                for mt in range(NMT):
                    t0 = mt * 512
                    xt = xp.tile([128, 4 * 512], BF16, tag="xt")
                    nc.sync.dma_start(
                        out=xt[:].rearrange("p (k t) -> p k t", k=4),
                        in_=xin[b, :, t0:t0 + 512].rearrange("(k p) t -> p k t", p=128))
                    xts.append(xt)
                for ob in range(4):
                    for mt in range(NMT):
                        t0 = mt * 512
                        for (wt, c, bcol) in ((brg_t, 0, C_BRG), (big_t, 1, C_BIG)):
                            pm = ps1.tile([128, 512], F32, tag="pm1")
                            for kt in range(4):
                                nc.tensor.matmul(
                                    pm[:], wt[:, kt * 512 + ob * 128:kt * 512 + ob * 128 + 128],
                                    xts[mt][:, kt * 512:(kt + 1) * 512],
                                    start=(kt == 0), stop=(kt == 3))
                            nc.scalar.activation(
                                uplane(b, ob, c)[:, t0:t0 + 512], pm[:],
                                ACTF.Identity, bias=col(bcol, ob), scale=1.0)

            def rot_ops(b, bk, g):
                # rotate full row: v = e^{-i theta s} * u (in place); yields ops
                uR = uplane(b, bk, 0)
                uI = uplane(b, bk, 1)
                c_, s_ = ctab(bk), stab(bk)
                t1, t2, t3 = tmps[g]
                yield lambda: nc.vector.tensor_tensor(t1[:], c_, uR, ALU.mult)
                yield lambda: nc.vector.tensor_tensor(t2[:], s_, uR, ALU.mult)
                yield lambda: nc.vector.tensor_tensor(t3[:], s_, uI, ALU.mult)
                yield lambda: nc.vector.tensor_tensor(uR, t1[:], t3[:], ALU.add)
                yield lambda: nc.vector.tensor_tensor(t1[:], c_, uI, ALU.mult)
                yield lambda: nc.vector.tensor_tensor(uI, t1[:], t2[:], ALU.subtract)

            def half_ops(b, bk, hf, g):
                # scan half + unrotate half (in place); yields ops
                uR = uplane(b, bk, 0)
                uI = uplane(b, bk, 1)
                rho = rho_t[:, bk * SCH:(bk + 1) * SCH]
                s0 = hf * SCH
                sl = slice(s0, s0 + SCH)
                c_, s_ = ctab(bk)[:, sl], stab(bk)[:, sl]
                t1, t2, t3 = tmps[g]
                for ci, pl in ((0, uR), (1, uI)):
                    ini = 0.0 if hf == 0 else carry[:, bk * 2 + ci:bk * 2 + ci + 1]
                    yield lambda pl=pl, ini=ini: nc.vector.tensor_tensor_scan(
                        pl[:, sl], rho, pl[:, sl], ini, ALU.mult, ALU.add)
                if hf == 0:
                    yield lambda: nc.vector.tensor_copy(
                        carry[:, bk * 2:bk * 2 + 1], uR[:, s0 + SCH - 1:s0 + SCH])
                    yield lambda: nc.vector.tensor_copy(
                        carry[:, bk * 2 + 1:bk * 2 + 2], uI[:, s0 + SCH - 1:s0 + SCH])
                yield lambda: nc.vector.tensor_tensor(t1[:, :SCH], c_, uR[:, sl], ALU.mult)
                yield lambda: nc.vector.tensor_tensor(t2[:, :SCH], s_, uR[:, sl], ALU.mult)
                yield lambda: nc.vector.tensor_tensor(t3[:, :SCH], s_, uI[:, sl], ALU.mult)
                yield lambda: nc.vector.tensor_tensor(uR[:, sl], t1[:, :SCH], t3[:, :SCH], ALU.subtract)
                yield lambda: nc.vector.tensor_tensor(t1[:, :SCH], c_, uI[:, sl], ALU.mult)
                yield lambda: nc.vector.tensor_tensor(uI[:, sl], t2[:, :SCH], t1[:, :SCH], ALU.add)

            def interleave(*streams):
                streams = [iter(s) for s in streams]
                while streams:
                    nxt = []
                    for s in streams:
                        try:
                            next(s)()
                            nxt.append(s)
                        except StopIteration:
                            pass
                    streams = nxt

            def chain(*gens):
                for gn in gens:
                    yield from gn

            p2_state = {}

            def p2_front(b, mt):
                t0 = mt * 512
                ys = []
                y2s = []
                for ob in range(4):
                    p2 = psy.tile([128, 512], F32, tag="py")
                    for bk in range(4):
                        nc.tensor.matmul(
                            p2[:], cr_t[:, bk * 512 + ob * 128:bk * 512 + ob * 128 + 128],
                            uplane(b, bk, 0)[:, t0:t0 + 512],
                            start=(bk == 0), stop=False)
                    for bk in range(4):
                        nc.tensor.matmul(
                            p2[:], ci_t[:, bk * 512 + ob * 128:bk * 512 + ob * 128 + 128],
                            uplane(b, bk, 1)[:, t0:t0 + 512],
                            start=False, stop=(bk == 3))
                    y = yp.tile([128, 512], BF16, tag="y", name=f"y{ob}")
                    y2 = y2p.tile([128, 512], BF16, tag="y2", name=f"y2_{ob}")
                    nc.scalar.activation(y[:], p2[:], ACTF.Identity,
                                         bias=col(C_CRCI, ob), scale=1.0)
                    nc.scalar.activation(y2[:], p2[:], ACTF.Square,
                                         bias=col(C_CRCI, ob), scale=1.0)
                    ys.append(y)
                    y2s.append(y2)
                # per-token stats [1, 512]
                s1 = pst.tile([1, 512], F32, tag="s1", name="s1")
                s2 = pst.tile([1, 512], F32, tag="s2", name="s2")
                for ob in range(4):
                    nc.tensor.matmul(s1[:], ones_t[:, 0:1], ys[ob][:],
                                     start=(ob == 0), stop=(ob == 3))
                for ob in range(4):
                    nc.tensor.matmul(s2[:], ones_t[:, 0:1], y2s[ob][:],
                                     start=(ob == 0), stop=(ob == 3))
                d1 = stp.tile([1, 512], F32, tag="d1")
                d2 = stp.tile([1, 512], F32, tag="d2")
                ms = stp.tile([1, 512], F32, tag="ms")
                vps = stp.tile([1, 512], F32, tag="vps")
                inv = stp.tile([1, 512], F32, tag="inv")
                A1 = stp.tile([1, 512], F32, tag="A1")
                A1t = a1p.tile([128, 4], F32, tag="A1t")
                nc.scalar.activation(d1[:], s1[:], ACTF.Copy, scale=1.0 / H)
                nc.scalar.activation(d2[:], s2[:], ACTF.Identity, scale=1.0 / H,
                                     bias=eps_t[0:1, :])
                nc.vector.tensor_tensor(ms[:], d1[:], d1[:], ALU.mult)
                nc.vector.scalar_tensor_tensor(vps[:], ms[:], -1.0, d2[:],
                                               ALU.mult, ALU.add)
                nc.vector.reciprocal_approx_fast(inv[:], vps[:])
                nc.scalar.activation(A1[:], inv[:], ACTF.Sqrt)
                # transpose A1 [1,512] -> [128,4] via PE: col tb = A1-slice^T @ [1]
                pa = ps4.tile([128, 512], F32, tag="p4", name="pa1t")
                for tb in range(4):
                    nc.tensor.matmul(pa[:, tb:tb + 1],
                                     A1[:, tb * 128:(tb + 1) * 128],
                                     ones32[:], start=True, stop=True)
                nc.scalar.activation(A1t[:], pa[:, 0:4], ACTF.Copy)
                p2_state[(b, mt)] = (ys, A1t)

            def p2_back(b, mt):
                t0 = mt * 512
                ys, A1t = p2_state.pop((b, mt))
                # MLP collapsed + LN fold: p4t[t, o] = sum_k y[k,t] * W12c[k,o]
                for tb in range(4):
                    p4 = ps4.tile([128, 512], F32, tag="p4")
                    for kt in range(4):
                        nc.tensor.matmul(
                            p4[:], ys[kt][:, tb * 128:(tb + 1) * 128],
                            w12_t[:, kt * 512:(kt + 1) * 512],
                            start=(kt == 0), stop=(kt == 3))
                    outf = ofp.tile([128, 512], F32, tag="outf")
                    nc.scalar.activation(outf[:], p4[:], ACTF.Copy,
                                         scale=A1t[:, tb:tb + 1])
                    nc.sync.dma_start(
                        out=out[b, t0 + tb * 128:t0 + (tb + 1) * 128, :],
                        in_=outf[:])

            # ---- emission order (pipelining) ----
            mm1(0)
            # tables after mm1(0) DMAs, per-bk so bk0's tables land first
            for bk in range(4):
                nc.sync.dma_start(
                    out=tab_t[:, 2 * bk * T:(2 * bk + 2) * T].rearrange(
                        "p (g t) -> p g t", g=2),
                    in_=TABS[2 * bk * 128:(2 * bk + 2) * 128, :].rearrange(
                        "(g p) t -> p g t", p=128))
                nc.sync.dma_start(
                    out=rho_t[:, bk * SCH:(bk + 1) * SCH],
                    in_=RHO[bk * 128:(bk + 1) * 128, :])
            interleave(chain(rot_ops(0, 0, 0), half_ops(0, 0, 0, 0)),
                       chain(rot_ops(0, 1, 1), half_ops(0, 1, 0, 1)))
            interleave(chain(rot_ops(0, 2, 0), half_ops(0, 2, 0, 0)),
                       chain(rot_ops(0, 3, 1), half_ops(0, 3, 0, 1)))
            mm1(1)
            for (dst, src) in ((cr_t, Crt), (ci_t, Cin), (w12_t, W12)):
                nc.sync.dma_start(
                    out=dst[:].rearrange("p (k n) -> p k n", k=4),
                    in_=src[:].rearrange("(k p) n -> p k n", p=128))
            interleave(half_ops(0, 0, 1, 0), half_ops(0, 1, 1, 1))
            interleave(half_ops(0, 2, 1, 0), half_ops(0, 3, 1, 1))
            # b1 units interleaved with pipelined phase2(0)
            interleave(chain(rot_ops(1, 0, 0), half_ops(1, 0, 0, 0)),
                       chain(rot_ops(1, 1, 1), half_ops(1, 1, 0, 1)))
            p2_front(0, 0)
            p2_front(0, 1)
            p2_back(0, 0)
            interleave(chain(rot_ops(1, 2, 0), half_ops(1, 2, 0, 0)),
                       chain(rot_ops(1, 3, 1), half_ops(1, 3, 0, 1)))
            p2_front(0, 2)
            p2_back(0, 1)
            p2_front(0, 3)
            p2_back(0, 2)
            interleave(half_ops(1, 0, 1, 0), half_ops(1, 1, 1, 1))
            p2_back(0, 3)
            p2_front(1, 0)
            interleave(half_ops(1, 2, 1, 0), half_ops(1, 3, 1, 1))
            p2_front(1, 1)
            p2_back(1, 0)
            p2_front(1, 2)
            p2_back(1, 1)
            p2_front(1, 3)
            p2_back(1, 2)
            p2_back(1, 3)

    nc.compile()
    return nc


def _consts(nu_log, theta_log, gamma_log, br, bi, cr, ci, ln_scale, ln_bias,
            W1, b1, W2, b2):
    nu = np.exp(nu_log.astype(np.float64))
    theta = np.exp(theta_log.astype(np.float64))
    rho = np.exp(-nu)                       # |lambda|
    gamma = np.exp(gamma_log.astype(np.float64))
    W1s = W1.astype(np.float64) * ln_scale.astype(np.float64)[:, None]
    W12 = W1s @ W2.astype(np.float64)
    col6 = W12.sum(0)                        # ln_scale @ W1 @ W2
    # fold -mean*col6 into the weights: W12c = W12 - ones*col6/H
    W12c = W12 - col6[None, :] / H
    cols7 = ((ln_bias.astype(np.float64) @ W1.astype(np.float64)
              + b1.astype(np.float64)) @ W2.astype(np.float64)
             + b2.astype(np.float64)).astype(np.float32)
    cols = {}
    cols[C_BRG] = br.astype(np.float64) * gamma
    cols[C_BIG] = bi.astype(np.float64) * gamma
    cols[C_CRCI] = (cr - ci).astype(np.float64)
    cst = np.zeros((128, 4 * 3), np.float32)
    for c, v in cols.items():
        for blk in range(4):
            cst[:, c * 4 + blk] = v[blk * 128:(blk + 1) * 128].astype(np.float32)
    # twiddle tables: per Bk block, cos/sin(theta_h * t), [8*128, T]
    t_idx = np.arange(T, dtype=np.float64)
    ang = theta[:, None] * t_idx[None, :]          # [H, T]
    bf = ml_dtypes.bfloat16
    tabs = np.zeros((8 * 128, T), bf)
    for blk in range(4):
        hs = slice(blk * 128, (blk + 1) * 128)
        tabs[2 * blk * 128:(2 * blk + 1) * 128] = np.cos(ang[hs]).astype(bf)
        tabs[(2 * blk + 1) * 128:(2 * blk + 2) * 128] = np.sin(ang[hs]).astype(bf)
    rho_tab = np.repeat(rho.astype(np.float32)[:, None], SCH, axis=1)  # [512, SCH]
    return cst, tabs, rho_tab, gamma, W12c, cols7


def kernel(x, nu_log, theta_log, gamma_log, Br, br, Bi, bi,
           Cr, cr, Ci, ci, ln_scale, ln_bias, W1, b1, W2, b2):
    if "nc" not in _CACHE:
        _CACHE["nc"] = _build()
    nc = _CACHE["nc"]
    cst, tabs, rho_tab, gamma, W12c, cols7 = _consts(
        nu_log, theta_log, gamma_log, br, bi, cr, ci,
        ln_scale, ln_bias, W1, b1, W2, b2)
    bf = ml_dtypes.bfloat16
    g32 = gamma.astype(np.float32)
    Brg = (Br * g32[None, :]).astype(bf)
    Big = (Bi * g32[None, :]).astype(bf)
    Crb = Cr.astype(bf)
    Cinb = (-Ci).astype(bf)
    W12b = W12c.astype(np.float32).astype(bf)
    xt = np.ascontiguousarray(x.transpose(0, 2, 1)).astype(bf)  # [B, H, T]
    in_maps = []
    for i in range(NCORES):
        in_maps.append(dict(x_t=xt[2 * i:2 * i + 2], Brg=Brg, Big=Big,
                            Crt=Crb, Cin=Cinb, W12=W12b, tabs=tabs,
                            rho=rho_tab, cst=cst))
    res = run_bass_kernel_spmd(nc, in_maps, core_ids=list(range(NCORES)))
    out = np.empty((B, T, O), np.float32)
    for i in range(NCORES):
        out[2 * i:2 * i + 2] = res.results[i]["out_t"]  # [NB, T, O]
    if np.any(cols7):
        out += cols7[None, None, :]
    return out


# revision 26
# speedup vs baseline: 1.0954x; 1.0954x over previous
import sys
sys.path.insert(0, '/opt/trn_rl_repo')
import numpy as np
import ml_dtypes
import concourse.bacc as bacc
import concourse.mybir as mybir
import concourse.tile as tile
from concourse.bass_utils import run_bass_kernel_spmd

F32 = mybir.dt.float32
BF16 = mybir.dt.bfloat16
ALU = mybir.AluOpType
ACTF = mybir.ActivationFunctionType

B, T, H, O = 16, 2048, 512, 512
NB = 2            # batch rows per core
NCORES = 8
NMT = T // 512    # 512-token tiles per row
SCH = 1024        # scan chunk length (= half of T)
LN_EPS = 1e-6

_CACHE = {}

# cst column layout: 3 consts x 4 blocks
C_BRG, C_BIG, C_CRCI = range(3)


def _build():
    nc = bacc.Bacc(None, target_bir_lowering=False)
    xin = nc.declare_dram_parameter("x_t", [NB, H, T], BF16, False)
    Brg = nc.declare_dram_parameter("Brg", [H, H], BF16, False)
    Big = nc.declare_dram_parameter("Big", [H, H], BF16, False)
    Crt = nc.declare_dram_parameter("Crt", [H, H], BF16, False)
    Cin = nc.declare_dram_parameter("Cin", [H, H], BF16, False)
    W12 = nc.declare_dram_parameter("W12", [H, H], BF16, False)
    TABS = nc.declare_dram_parameter("tabs", [8 * 128, T], BF16, False)
    RHO = nc.declare_dram_parameter("rho", [4 * 128, SCH], F32, False)
    CST = nc.declare_dram_parameter("cst", [128, 4 * 3], F32, False)
    out = nc.declare_dram_parameter("out_t", [NB, T, O], F32, True)

    with tile.TileContext(nc) as tc:
        with tc.tile_pool(name="wpool", bufs=1) as wp, \
             tc.tile_pool(name="upool", bufs=1) as up, \
             tc.tile_pool(name="tmpp", bufs=1) as tp, \
             tc.tile_pool(name="xp", bufs=5) as xp, \
             tc.tile_pool(name="yp", bufs=8) as yp, \
             tc.tile_pool(name="y2p", bufs=4) as y2p, \
             tc.tile_pool(name="stp", bufs=1) as stp, \
             tc.tile_pool(name="a1p", bufs=2) as a1p, \
             tc.tile_pool(name="ofp", bufs=3) as ofp, \
             tc.tile_pool(name="ps_mm1", bufs=2, space="PSUM") as ps1, \
             tc.tile_pool(name="ps_y", bufs=2, space="PSUM") as psy, \
             tc.tile_pool(name="ps_st", bufs=1, space="PSUM") as pst, \
             tc.tile_pool(name="ps_p4", bufs=2, space="PSUM") as ps4:

            # ---- early weights (mm1 path) ----
            brg_t = wp.tile([128, 4 * 512], BF16, tag="brg")
            big_t = wp.tile([128, 4 * 512], BF16, tag="big")
            cst_t = wp.tile([128, 4 * 3], F32, tag="cst")
            for (dst, src) in ((brg_t, Brg), (big_t, Big)):
                nc.sync.dma_start(
                    out=dst[:].rearrange("p (k n) -> p k n", k=4),
                    in_=src[:].rearrange("(k p) n -> p k n", p=128))
            nc.sync.dma_start(out=cst_t[:], in_=CST[:])

            cr_t = wp.tile([128, 4 * 512], BF16, tag="cr")
            ci_t = wp.tile([128, 4 * 512], BF16, tag="ci")
            w12_t = wp.tile([128, 4 * 512], BF16, tag="w12")
            tab_t = wp.tile([128, 8 * T], BF16, tag="tabs")
            rho_t = wp.tile([128, 4 * SCH], F32, tag="rho")
            ones_t = wp.tile([128, 128], BF16, tag="ones")
            ones32 = wp.tile([1, 1], F32, tag="ones32")
            mones32 = wp.tile([1, 1], F32, tag="mones32")
            eps_t = wp.tile([128, 1], F32, tag="eps")
            nc.vector.memset(ones_t[:], 1.0)
            nc.vector.memset(ones32[:], 1.0)
            nc.vector.memset(mones32[:], -1.0)
            nc.vector.memset(eps_t[:], LN_EPS)

            def col(c, blk):
                return cst_t[:, c * 4 + blk:c * 4 + blk + 1]

            def ctab(bk):
                return tab_t[:, (2 * bk) * T:(2 * bk + 1) * T]

            def stab(bk):
                return tab_t[:, (2 * bk + 1) * T:(2 * bk + 2) * T]

            # u/h storage: per (b, Bk): R and I planes, token-contiguous
            U = up.tile([128, NB * 4 * 2 * T], BF16, tag="u")
            uv = U[:].rearrange("p (b k c t) -> p b k c t", b=NB, k=4, c=2)

            def uplane(b, bk, c):
                return uv[:, b:b + 1, bk:bk + 1, c:c + 1, :].squeeze()

            tmps = [[tp.tile([128, T], BF16, tag=f"tmp{j}_{g}", name=f"tmp{j}_{g}")
                     for j in range(3)] for g in range(2)]
            carry = tp.tile([128, 8], BF16, tag="carry")

            def mm1(b):
                xts = []
                for mt in range(NMT):
                    t0 = mt * 512
                    xt = xp.tile([128, 4 * 512], BF16, tag="xt")
                    nc.sync.dma_start(
                        out=xt[:].rearrange("p (k t) -> p k t", k=4),
                        in_=xin[b, :, t0:t0 + 512].rearrange("(k p) t -> p k t", p=128))
                    xts.append(xt)
                for ob in range(4):
                    for mt in range(NMT):
                        t0 = mt * 512
                        for (wt, c, bcol) in ((brg_t, 0, C_BRG), (big_t, 1, C_BIG)):
                            pm = ps1.tile([128, 512], F32, tag="pm1")
                            for kt in range(4):
                                nc.tensor.matmul(
                                    pm[:], wt[:, kt * 512 + ob * 128:kt * 512 + ob * 128 + 128],
                                    xts[mt][:, kt * 512:(kt + 1) * 512],
                                    start=(kt == 0), stop=(kt == 3))
                            nc.scalar.activation(
                                uplane(b, ob, c)[:, t0:t0 + 512], pm[:],
                                ACTF.Identity, bias=col(bcol, ob), scale=1.0)

            def rot_ops(b, bk, g):
                # rotate full row: v = e^{-i theta s} * u (in place); yields ops
                uR = uplane(b, bk, 0)
                uI = uplane(b, bk, 1)
                c_, s_ = ctab(bk), stab(bk)
                t1, t2, t3 = tmps[g]
                yield lambda: nc.vector.tensor_tensor(t1[:], c_, uR, ALU.mult)
                yield lambda: nc.vector.tensor_tensor(t2[:], s_, uR, ALU.mult)
                yield lambda: nc.vector.tensor_tensor(t3[:], s_, uI, ALU.mult)
                yield lambda: nc.vector.tensor_tensor(uR, t1[:], t3[:], ALU.add)
                yield lambda: nc.vector.tensor_tensor(t1[:], c_, uI, ALU.mult)
                yield lambda: nc.vector.tensor_tensor(uI, t1[:], t2[:], ALU.subtract)

            def half_ops(b, bk, hf, g):
                # scan half + unrotate half (in place); yields ops
                uR = uplane(b, bk, 0)
                uI = uplane(b, bk, 1)
                rho = rho_t[:, bk * SCH:(bk + 1) * SCH]
                s0 = hf * SCH
                sl = slice(s0, s0 + SCH)
                c_, s_ = ctab(bk)[:, sl], stab(bk)[:, sl]
                t1, t2, t3 = tmps[g]
                for ci, pl in ((0, uR), (1, uI)):
                    ini = 0.0 if hf == 0 else carry[:, bk * 2 + ci:bk * 2 + ci + 1]
                    yield lambda pl=pl, ini=ini: nc.vector.tensor_tensor_scan(
                        pl[:, sl], rho, pl[:, sl], ini, ALU.mult, ALU.add)
                if hf == 0:
                    yield lambda: nc.vector.tensor_copy(
                        carry[:, bk * 2:bk * 2 + 1], uR[:, s0 + SCH - 1:s0 + SCH])
                    yield lambda: nc.vector.tensor_copy(
                        carry[:, bk * 2 + 1:bk * 2 + 2], uI[:, s0 + SCH - 1:s0 + SCH])
                yield lambda: nc.vector.tensor_tensor(t1[:, :SCH], c_, uR[:, sl], ALU.mult)
                yield lambda: nc.vector.tensor_tensor(t2[:, :SCH], s_, uR[:, sl], ALU.mult)
                yield lambda: nc.vector.tensor_tensor(t3[:, :SCH], s_, uI[:, sl], ALU.mult)
                yield lambda: nc.vector.tensor_tensor(uR[:, sl], t1[:, :SCH], t3[:, :SCH], ALU.subtract)
                yield lambda: nc.vector.tensor_tensor(t1[:, :SCH], c_, uI[:, sl], ALU.mult)
                yield lambda: nc.vector.tensor_tensor(uI[:, sl], t2[:, :SCH], t1[:, :SCH], ALU.add)

            def interleave(*streams):
                streams = [iter(s) for s in streams]
                while streams:
                    nxt = []
                    for s in streams:
                        try:
                            next(s)()
                            nxt.append(s)
                        except StopIteration:
                            pass
                    streams = nxt

            def chain(*gens):
                for gn in gens:
                    yield from gn

            p2_state = {}

            def p2_front(b, mt):
                t0 = mt * 512
                ys = []
                y2s = []
                for ob in range(4):
                    p2 = psy.tile([128, 512], F32, tag="py")
                    for bk in range(4):
                        nc.tensor.matmul(
                            p2[:], cr_t[:, bk * 512 + ob * 128:bk * 512 + ob * 128 + 128],
                            uplane(b, bk, 0)[:, t0:t0 + 512],
                            start=(bk == 0), stop=False)
                    for bk in range(4):
                        nc.tensor.matmul(
                            p2[:], ci_t[:, bk * 512 + ob * 128:bk * 512 + ob * 128 + 128],
                            uplane(b, bk, 1)[:, t0:t0 + 512],
                            start=False, stop=(bk == 3))
                    y = yp.tile([128, 512], BF16, tag="y", name=f"y{ob}")
                    y2 = y2p.tile([128, 512], BF16, tag="y2", name=f"y2_{ob}")
                    nc.scalar.activation(y[:], p2[:], ACTF.Identity,
                                         bias=col(C_CRCI, ob), scale=1.0)
                    nc.scalar.activation(y2[:], p2[:], ACTF.Square,
                                         bias=col(C_CRCI, ob), scale=1.0)
                    ys.append(y)
                    y2s.append(y2)
                # per-token stats [1, 512]
                s1 = pst.tile([1, 512], F32, tag="s1", name="s1")
                s2 = pst.tile([1, 512], F32, tag="s2", name="s2")
                for ob in range(4):
                    nc.tensor.matmul(s1[:], ones_t[:, 0:1], ys[ob][:],
                                     start=(ob == 0), stop=(ob == 3))
                for ob in range(4):
                    nc.tensor.matmul(s2[:], ones_t[:, 0:1], y2s[ob][:],
                                     start=(ob == 0), stop=(ob == 3))
                d1 = stp.tile([1, 512], F32, tag="d1")
                d2 = stp.tile([1, 512], F32, tag="d2")
                ms = stp.tile([1, 512], F32, tag="ms")
                lnv = stp.tile([1, 512], F32, tag="lnv")
                A1 = stp.tile([1, 512], F32, tag="A1")
                A1t = a1p.tile([128, 4], F32, tag="A1t")
                nc.scalar.activation(d1[:], s1[:], ACTF.Copy, scale=1.0 / H)
                nc.scalar.activation(d2[:], s2[:], ACTF.Identity, scale=1.0 / H,
                                     bias=eps_t[0:1, :])
                nc.scalar.activation(ms[:], d1[:], ACTF.Square)
                # vps = d2 - ms via two k=1 matmuls (keeps the chain off the DVE)
                vps = pst.tile([1, 512], F32, tag="s2", name="vps")
                nc.tensor.matmul(vps[:], ones32[:], d2[:], start=True, stop=False)
                nc.tensor.matmul(vps[:], mones32[:], ms[:], start=False, stop=True)
                # A1 = 1/sqrt(vps) = exp(-0.5 * ln(vps))
                nc.scalar.activation(lnv[:], vps[:], ACTF.Ln)
                nc.scalar.activation(A1[:], lnv[:], ACTF.Exp, scale=-0.5)
                # transpose A1 [1,512] -> [128,4] via PE: col tb = A1-slice^T @ [1]
                pa = ps4.tile([128, 512], F32, tag="p4", name="pa1t")
                for tb in range(4):
                    nc.tensor.matmul(pa[:, tb:tb + 1],
                                     A1[:, tb * 128:(tb + 1) * 128],
                                     ones32[:], start=True, stop=True)
                nc.scalar.activation(A1t[:], pa[:, 0:4], ACTF.Copy)
                p2_state[(b, mt)] = (ys, A1t)

            def p2_back(b, mt):
                t0 = mt * 512
                ys, A1t = p2_state.pop((b, mt))
                # MLP collapsed + LN fold: p4t[t, o] = sum_k y[k,t] * W12c[k,o]
                for tb in range(4):
                    p4 = ps4.tile([128, 512], F32, tag="p4")
                    for kt in range(4):
                        nc.tensor.matmul(
                            p4[:], ys[kt][:, tb * 128:(tb + 1) * 128],
                            w12_t[:, kt * 512:(kt + 1) * 512],
                            start=(kt == 0), stop=(kt == 3))
                    outf = ofp.tile([128, 512], F32, tag="outf")
                    nc.scalar.activation(outf[:], p4[:], ACTF.Copy,
                                         scale=A1t[:, tb:tb + 1])
                    nc.sync.dma_start(
                        out=out[b, t0 + tb * 128:t0 + (tb + 1) * 128, :],
                        in_=outf[:])

            # ---- emission order (pipelining) ----
            mm1(0)
            # tables after mm1(0) DMAs, per-bk so bk0's tables land first
            for bk in range(4):
                nc.sync.dma_start(
                    out=tab_t[:, 2 * bk * T:(2 * bk + 2) * T].rearrange(
                        "p (g t) -> p g t", g=2),
                    in_=TABS[2 * bk * 128:(2 * bk + 2) * 128, :].rearrange(
                        "(g p) t -> p g t", p=128))
                nc.sync.dma_start(
                    out=rho_t[:, bk * SCH:(bk + 1) * SCH],
                    in_=RHO[bk * 128:(bk + 1) * 128, :])
            interleave(chain(rot_ops(0, 0, 0), half_ops(0, 0, 0, 0)),
                       chain(rot_ops(0, 1, 1), half_ops(0, 1, 0, 1)))
            interleave(chain(rot_ops(0, 2, 0), half_ops(0, 2, 0, 0)),
                       chain(rot_ops(0, 3, 1), half_ops(0, 3, 0, 1)))
            mm1(1)
            for (dst, src) in ((cr_t, Crt), (ci_t, Cin), (w12_t, W12)):
                nc.sync.dma_start(
                    out=dst[:].rearrange("p (k n) -> p k n", k=4),
                    in_=src[:].rearrange("(k p) n -> p k n", p=128))
            interleave(half_ops(0, 0, 1, 0), half_ops(0, 1, 1, 1))
            interleave(half_ops(0, 2, 1, 0), half_ops(0, 3, 1, 1))
            # b1 units interleaved with pipelined phase2(0)
            interleave(chain(rot_ops(1, 0, 0), half_ops(1, 0, 0, 0)),
                       chain(rot_ops(1, 1, 1), half_ops(1, 1, 0, 1)))
            p2_front(0, 0)
            p2_front(0, 1)
            p2_back(0, 0)
            interleave(chain(rot_ops(1, 2, 0), half_ops(1, 2, 0, 0)),
                       chain(rot_ops(1, 3, 1), half_ops(1, 3, 0, 1)))
            p2_front(0, 2)
            p2_back(0, 1)
            p2_front(0, 3)
            p2_back(0, 2)
            interleave(half_ops(1, 0, 1, 0), half_ops(1, 1, 1, 1))
            p2_back(0, 3)
            p2_front(1, 0)
            interleave(half_ops(1, 2, 1, 0), half_ops(1, 3, 1, 1))
            p2_front(1, 1)
            p2_back(1, 0)
            p2_front(1, 2)
            p2_back(1, 1)
            p2_front(1, 3)
            p2_back(1, 2)
            p2_back(1, 3)

    nc.compile()
    return nc


def _consts(nu_log, theta_log, gamma_log, br, bi, cr, ci, ln_scale, ln_bias,
            W1, b1, W2, b2):
    nu = np.exp(nu_log.astype(np.float64))
    theta = np.exp(theta_log.astype(np.float64))
    rho = np.exp(-nu)                       # |lambda|
    gamma = np.exp(gamma_log.astype(np.float64))
    W1s = W1.astype(np.float64) * ln_scale.astype(np.float64)[:, None]
    W12 = W1s @ W2.astype(np.float64)
    col6 = W12.sum(0)                        # ln_scale @ W1 @ W2
    # fold -mean*col6 into the weights: W12c = W12 - ones*col6/H
    W12c = W12 - col6[None, :] / H
    cols7 = ((ln_bias.astype(np.float64) @ W1.astype(np.float64)
              + b1.astype(np.float64)) @ W2.astype(np.float64)
             + b2.astype(np.float64)).astype(np.float32)
    cols = {}
    cols[C_BRG] = br.astype(np.float64) * gamma
    cols[C_BIG] = bi.astype(np.float64) * gamma
    cols[C_CRCI] = (cr - ci).astype(np.float64)
    cst = np.zeros((128, 4 * 3), np.float32)
    for c, v in cols.items():
        for blk in range(4):
            cst[:, c * 4 + blk] = v[blk * 128:(blk + 1) * 128].astype(np.float32)
    # twiddle tables: per Bk block, cos/sin(theta_h * t), [8*128, T]
    t_idx = np.arange(T, dtype=np.float64)
    ang = theta[:, None] * t_idx[None, :]          # [H, T]
    bf = ml_dtypes.bfloat16
    tabs = np.zeros((8 * 128, T), bf)
    for blk in range(4):
        hs = slice(blk * 128, (blk + 1) * 128)
        tabs[2 * blk * 128:(2 * blk + 1) * 128] = np.cos(ang[hs]).astype(bf)
        tabs[(2 * blk + 1) * 128:(2 * blk + 2) * 128] = np.sin(ang[hs]).astype(bf)
    rho_tab = np.repeat(rho.astype(np.float32)[:, None], SCH, axis=1)  # [512, SCH]
    return cst, tabs, rho_tab, gamma, W12c, cols7


def kernel(x, nu_log, theta_log, gamma_log, Br, br, Bi, bi,
           Cr, cr, Ci, ci, ln_scale, ln_bias, W1, b1, W2, b2):
    if "nc" not in _CACHE:
        _CACHE["nc"] = _build()
    nc = _CACHE["nc"]
    cst, tabs, rho_tab, gamma, W12c, cols7 = _consts(
        nu_log, theta_log, gamma_log, br, bi, cr, ci,
        ln_scale, ln_bias, W1, b1, W2, b2)
    bf = ml_dtypes.bfloat16
    g32 = gamma.astype(np.float32)
    Brg = (Br * g32[None, :]).astype(bf)
    Big = (Bi * g32[None, :]).astype(bf)
    Crb = Cr.astype(bf)
    Cinb = (-Ci).astype(bf)
    W12b = W12c.astype(np.float32).astype(bf)
    xt = np.ascontiguousarray(x.transpose(0, 2, 1)).astype(bf)  # [B, H, T]
    in_maps = []
    for i in range(NCORES):
        in_maps.append(dict(x_t=xt[2 * i:2 * i + 2], Brg=Brg, Big=Big,
                            Crt=Crb, Cin=Cinb, W12=W12b, tabs=tabs,
                            rho=rho_tab, cst=cst))
    res = run_bass_kernel_spmd(nc, in_maps, core_ids=list(range(NCORES)))
    out = np.empty((B, T, O), np.float32)
    for i in range(NCORES):
        out[2 * i:2 * i + 2] = res.results[i]["out_t"]  # [NB, T, O]
    if np.any(cols7):
        out += cols7[None, None, :]
    return out
